# revision 2
# baseline (speedup 1.0000x reference)
"""Trainium2 Bass kernel for nn_DiscriminationLoss (segment_reduce), v2.

Math: per image b the loss reduces to
  S[b,k,c]    = sum of pred[b,c] over pixels with label k   (k=1..16)
  counts[b,k] = histogram of labels
followed by a tiny scalar epilogue (norms, log-relu pair combination).

Device strategy (2 images per core, data-parallel over batch):
- Pixels: [128 partitions, 3200 pixel-cols] per image.  All step
  functionals S>=k = sum_{lab>=k} pred come from PE matmuls against
  per-pixel mask planes; S_k = S>=k - S>=k+1 on the host.
- bf16 main pass: stationary slab interleaved per 8-pixel-col group as
  11 slots x 8 chunks = 88 cols:
    slot 0  = all-ones plane  -> S>=1  (pred is host-zeroed on
              background, so the sum over ALL pixels = sum over lab>=1).
              Built by gpsimd/Pool MEMSET - zero DVE/Act cost.
    slots 1..10 = [lab >= k], k=2..11, built by DVE tensor_scalar
              (is_ge) in 4x perf mode, with free per-partition count
              accumulation (accum_out).
- fp8 side pass: 5 sign planes Sign(lab-k+0.5) = +-1 for k=12..16, in
  an interleaved (group, ktile, k, i) fp8 arena consumed by PE
  DoubleRow matmuls (256-deep contraction at 0.5 cyc/row).  Column
  ranges are either Act-built (Sign activation, bf16 in / fp8 out) or
  host-shipped verbatim via DMA (+-1 is exact in fp8e4m3).
  Decode: F8 = sum(+-1 * pred) = 2*S>=k - S>=1.
- Counts: k=2..11 from DVE accum_out; k=1 and k>=12 host-side from the
  label histogram (the background mask is already computed on the host
  for pred zeroing, so counting is free there).
- pred is fp8e4m3, host-converted and background-zeroed; labels bf16.

Toolchain notes:
- walrus rejects multi-wait instructions and any wait on TensorScalar;
  the BIR is post-processed to move those onto Drain predecessors.
- Matmul stationary APs must have ONE free dim (two for DoubleRow with
  16B-aligned even strides) - hence the interleaved slab layouts.

Sharding: data-parallel over batch, 2 images per core, no collectives.
"""

import json

import numpy as np
import ml_dtypes

import concourse.bass as bass
import concourse.mybir as mybir
import concourse.tile as tile
import concourse.bass2jax as _b2j
from concourse.bass_utils import run_bass_kernel_spmd


def _split_multiwait_bir(bir_json: bytes) -> bytes:
    """walrus rejects instructions carrying more than one sync wait, and any
    wait riding a TensorScalarPtr; split extra waits onto single-wait
    same-engine Drain predecessors."""
    d = json.loads(bir_json)
    changed = False
    for fn in d.get("functions", []):
        for bb in fn.get("blocks", []):
            insts = bb.get("instructions", [])
            out = []
            for ins in insts:
                si = ins.get("sync_info") or {}
                waits = si.get("on_wait") or []
                keep = 0 if (waits and ins.get("opcode") == "TensorScalarPtr") \
                    else 1
                if len(waits) > keep:
                    changed = True
                    split = waits[:len(waits) - keep]
                    for wi, w in enumerate(split):
                        out.append(
                            {
                                "debug": ins.get("debug"),
                                "engine": ins["engine"],
                                "ins": [],
                                "is_reset_sema": False,
                                "name": f"{ins['name']}_w{wi}",
                                "opcode": "Drain",
                                "outs": [],
                                "sync_info": {"on_update": [], "on_wait": [w]},
                            }
                        )
                    si["on_wait"] = waits[len(waits) - keep:]
                out.append(ins)
            bb["instructions"] = out
    if not changed:
        return bir_json
    return json.dumps(d).encode()


_ORIG_COMPILE_BIR = _b2j.compile_bir_kernel


def _compile_bir_splitting_waits(bir_json, tmpdir, neff_name="file.neff"):
    return _ORIG_COMPILE_BIR(_split_multiwait_bir(bir_json), tmpdir, neff_name=neff_name)


_b2j.compile_bir_kernel = _compile_bir_splitting_waits

B, C, H, W = 16, 4, 640, 640
HW = H * W
P = 128
FD = HW // P               # 3200 pixel-cols per image
N_CORES = 8
IPC = B // N_CORES         # images per core
NCOL = IPC * FD            # 6400 pixel-cols per core
KMAX = 16
K1 = KMAX + 1
SIGMA_DIS = 3.0
F0 = float(np.log(SIGMA_DIS**2 + 1.0))

# ---------------- configuration ----------------
D = 10                      # DVE step slots, k = 2..11
NSLOT = 1 + D               # + ones slot
SLABW = NSLOT * 8           # 88 stationary cols per 8-pixel-col group
NF8 = 5                     # fp8 sign slots, k = 12..16
F8K0 = 2 + D                # first fp8 k (=12)
F8GW = 2 * NF8 * 16         # fp8 arena cols per 32-pixel-col group (=160)

# bf16 spans per image (pixel-col boundaries); each span = one slab ring
# tile, one DVE op per step slot, one Pool ones-memset.
# spans: (c0, c1, kind) with kind "dve" (device-built planes) or "bship"
# (host-shipped interleaved bf16 slab; PE-only, no DVE/Act work)
BF_SPANS = [
    [(0, 384, "dve"), (384, 1920, "dve"), (1920, 3200, "dve")],
    [(0, 640, "dve"), (640, 1920, "dve"), (1920, 2880, "dve"),
     (2880, 3136, "dve"), (3136, 3200, "dve")],
]
# label DMA slices per image (pixel-cols)
LAB_SLICES = [
    [(0, 384), (384, 1920), (1920, 3200)],
    [(0, 640), (640, 1920), (1920, 3200)],
]
# emit act-range DoubleRow matmuls after this span's bf matmuls
ACT_EMIT_SP = {0: 1, 1: 2}
# accums of these (img, span) and drains of these segs close late -> tail
LATE_SPANS = {(1, 2), (1, 3), (1, 4)}
LATE_SEGS = {("bf", 1, 1), ("bf", 1, 2), ("f8", 1, 0), ("f8", 1, 1)}
# fp8 ranges per image: (c0, c1, kind); 32-col aligned
F8_RANGES = [
    [(0, 1920, "act"), (1920, 3200, "ship")],
    [(0, 1408, "ship"), (1408, 3072, "act"), (3072, 3200, "ship")],
]
# psum segments per image for the bf16 pass (8-col group indices);
# boundaries must coincide with span ends
BF_SEGS = [
    [(0, 240), (240, 400)],
    [(0, 240), (240, 392), (392, 400)],
]
# psum segments per image for the fp8 pass (32-col group indices)
F8_SEGS = [
    [(0, 100)],
    [(0, 98), (98, 100)],
]
# pred DMA slices per image (pixel-cols)
PRED_SLICES = [
    [(0, 384), (384, 1920), (1920, 3200)],
    [(0, 640), (640, 1920), (1920, 3200)],
]
# DMA issue order: tokens (kind, img, idx); labels early, pred/ship behind
DMA_ORDER = [
    ("lab", 0, 0), ("lab", 0, 1), ("pred", 0, 0), ("lab", 0, 2),
    ("lab", 1, 0), ("pred", 0, 1), ("ship", 0, 0), ("lab", 1, 1),
    ("lab", 1, 2), ("pred", 0, 2), ("pred", 1, 0), ("ship", 1, 0),
    ("pred", 1, 1), ("ship", 1, 1), ("pred", 1, 2),
]

# optional config override for scheduling sweeps
import os as _os
if _os.environ.get("KCFG"):
    _cfg = json.loads(_os.environ["KCFG"])
    for _k, _v in _cfg.items():
        globals()[_k] = _v
    BF_SPANS = [[tuple(x) for x in img] for img in BF_SPANS]
    LAB_SLICES = [[tuple(x) for x in img] for img in LAB_SLICES]
    PRED_SLICES = [[tuple(x) for x in img] for img in PRED_SLICES]
    F8_RANGES = [[tuple(x) for x in img] for img in F8_RANGES]
    BF_SEGS = [[tuple(x) for x in img] for img in BF_SEGS]
    F8_SEGS = [[tuple(x) for x in img] for img in F8_SEGS]
    LATE_SPANS = {tuple(x) for x in LATE_SPANS}
    LATE_SEGS = {tuple(x) for x in LATE_SEGS}
    DMA_ORDER = [tuple(x) for x in DMA_ORDER]
    ACT_EMIT_SP = {int(k): v for k, v in ACT_EMIT_SP.items()}

LAST_BF = ("bf", IPC - 1, len(BF_SEGS[IPC - 1]) - 1)
LAST_F8 = ("f8", IPC - 1, len(F8_SEGS[IPC - 1]) - 1)

RUN_KWARGS = {}
LAST_RESULT = None
_NC_CACHE = []

BF16 = mybir.dt.bfloat16
FP8 = mybir.dt.float8e4
F32 = mybir.dt.float32
AF = mybir.ActivationFunctionType

SHIP_COLS = [[(c0, c1) for c0, c1, kind in F8_RANGES[i] if kind == "ship"]
             for i in range(IPC)]
SHIP_TOT = sum(c1 - c0 for i in range(IPC) for c0, c1 in SHIP_COLS[i])
SHIP_ARENA = SHIP_TOT * NF8
BSHIP_SPANS = [[(c0, c1) for c0, c1, kind in BF_SPANS[i] if kind == "bship"]
               for i in range(IPC)]
BSHIP_ARENA = sum((c1 - c0) // 8 * SLABW
                  for i in range(IPC) for c0, c1 in BSHIP_SPANS[i])

# --- out tensor layout: [early accums | early drains | tail] ---
# accums/drains that close late live in the tail region so the main
# out-DMA never waits on them.
_acc_cols = {}
DRAIN_AT = {}
_c = 0
for i in range(IPC):
    for sp in range(len(BF_SPANS[i])):
        if (i, sp) in LATE_SPANS or BF_SPANS[i][sp][2] != "dve":
            continue
        for j in range(1, NSLOT):
            _acc_cols[(i, j, sp)] = _c
            _c += 1
_c = (_c + 31) // 32 * 32
for i in range(IPC):
    for s in range(len(BF_SEGS[i])):
        if ("bf", i, s) not in LATE_SEGS:
            DRAIN_AT[("bf", i, s)] = _c
            _c += 32
    for s in range(len(F8_SEGS[i])):
        if ("f8", i, s) not in LATE_SEGS:
            DRAIN_AT[("f8", i, s)] = _c
            _c += 64
TAIL_FROM = _c
for i in range(IPC):
    for sp in range(len(BF_SPANS[i])):
        if (i, sp) not in LATE_SPANS or BF_SPANS[i][sp][2] != "dve":
            continue
        for j in range(1, NSLOT):
            _acc_cols[(i, j, sp)] = _c
            _c += 1
_c = (_c + 31) // 32 * 32
for kind, i, s in sorted(LATE_SEGS):
    DRAIN_AT[(kind, i, s)] = _c
    _c += 32 if kind == "bf" else 64
# pad the tail block to >=128 f32 cols so its DMA runs at full descriptor
# bandwidth (contiguous run >= 512B per partition)
if _c - TAIL_FROM < 128:
    _c = TAIL_FROM + 128
OUT_COLS = _c


def _build_nc():
    nc = bass.Bass("TRN2", target_bir_lowering=False, debug=False)
    pred_d = nc.dram_tensor("pred", [P, NCOL * C], FP8, kind="ExternalInput")
    lab_d = nc.dram_tensor("lab", [P, NCOL], BF16, kind="ExternalInput")
    ship_d = nc.dram_tensor("ship", [P, max(SHIP_ARENA, 64)], FP8,
                            kind="ExternalInput")
    bship_d = nc.dram_tensor("bship", [P, max(BSHIP_ARENA, 64)], BF16,
                             kind="ExternalInput")
    out_d = nc.dram_tensor("out", [P, OUT_COLS], F32, kind="ExternalOutput")

    with tile.TileContext(nc) as tc:
        with tc.tile_pool(name="pool", bufs=1) as pool, \
             tc.tile_pool(name="ps", bufs=1, space="PSUM") as pspool:
            pred_sb = pool.tile([P, NCOL * C], FP8, name="pred_sb")
            lab_sb = pool.tile([P, NCOL], BF16, name="lab_sb")
            acc = pool.tile([P, OUT_COLS], F32, name="acc")
            consts = pool.tile([P, NF8], F32, name="consts")
            for j in range(NF8):
                nc.gpsimd.memset(consts[:, j:j + 1], 0.5 - float(F8K0 + j))

            # fp8 arena tiles, one per range
            f8_tiles = {}
            for img in range(IPC):
                for ridx, (c0, c1, kind) in enumerate(F8_RANGES[img]):
                    f8_tiles[(img, ridx)] = pool.tile(
                        [P, (c1 - c0) * NF8], FP8, name=f"f8_{img}_{ridx}")
            # host-shipped bf16 slab tiles (flat DMA destinations)
            bship_tiles = {}
            for img in range(IPC):
                for bidx, (c0, c1) in enumerate(BSHIP_SPANS[img]):
                    bship_tiles[(img, bidx)] = pool.tile(
                        [P, (c1 - c0) // 8 * SLABW], BF16,
                        name=f"bs_{img}_{bidx}")

            # ---------------- DMA schedule ----------------
            ship_ranges = []   # (img, ridx) in ship order
            for img in range(IPC):
                for ridx, (c0, c1, kind) in enumerate(F8_RANGES[img]):
                    if kind == "ship":
                        ship_ranges.append((img, ridx))
            ship_offs = {}
            off = 0
            for img, ridx in ship_ranges:
                c0, c1, _ = F8_RANGES[img][ridx]
                ship_offs[(img, ridx)] = off
                off += (c1 - c0) * NF8
            ship_by_img = {}
            for img, ridx in ship_ranges:
                ship_by_img.setdefault(img, []).append(ridx)
            bship_offs = {}
            boff = 0
            for img in range(IPC):
                for bidx, (c0, c1) in enumerate(BSHIP_SPANS[img]):
                    bship_offs[(img, bidx)] = boff
                    boff += (c1 - c0) // 8 * SLABW

            for kind, img, idx in DMA_ORDER:
                base = img * FD
                if kind == "lab":
                    c0, c1 = LAB_SLICES[img][idx]
                    nc.sync.dma_start(lab_sb[:, base + c0:base + c1],
                                      lab_d[:, base + c0:base + c1])
                elif kind == "pred":
                    c0, c1 = PRED_SLICES[img][idx]
                    nc.sync.dma_start(
                        pred_sb[:, (base + c0) * 4:(base + c1) * 4],
                        pred_d[:, (base + c0) * 4:(base + c1) * 4])
                elif kind == "ship":
                    ridx = ship_by_img[img][idx]
                    t = f8_tiles[(img, ridx)]
                    o = ship_offs[(img, ridx)]
                    w = t.shape[1]
                    nc.sync.dma_start(t[:], ship_d[:, o:o + w])
                elif kind == "bslab":
                    t = bship_tiles[(img, idx)]
                    o = bship_offs[(img, idx)]
                    w = t.shape[1]
                    nc.sync.dma_start(t[:], bship_d[:, o:o + w])

            # moving views
            pred4 = pred_sb[:].rearrange("p (g m) -> p g m", m=32)
            pred8 = pred_sb[:].rearrange("p (g t m) -> p g t m", t=2, m=64)

            bf_ps = [[pspool.tile([P, 32], F32, name=f"bps_{i}_{s}")
                      for s in range(len(BF_SEGS[i]))] for i in range(IPC)]
            f8_ps = [[pspool.tile([P, 64], F32, name=f"fps_{i}_{s}")
                      for s in range(len(F8_SEGS[i]))] for i in range(IPC)]

            def bf_seg_of(img, g):
                for s, (lo, hi) in enumerate(BF_SEGS[img]):
                    if lo <= g < hi:
                        return s, lo, hi
                raise AssertionError

            def f8_seg_of(img, g):
                for s, (lo, hi) in enumerate(F8_SEGS[img]):
                    if lo <= g < hi:
                        return s, lo, hi
                raise AssertionError

            # fp8 matmul bookkeeping: start/stop flags follow EMISSION order
            # (PE executes in order), so count totals per (img, seg) first.
            f8_tot = {}
            f8_cnt = {}
            for img in range(IPC):
                for s, (lo, hi) in enumerate(F8_SEGS[img]):
                    f8_tot[(img, s)] = hi - lo
                    f8_cnt[(img, s)] = 0

            def emit_f8_group(img, ridx, gl):
                r0, r1, kind = F8_RANGES[img][ridx]
                t = f8_tiles[(img, ridx)]
                nb = (r1 - r0) // 32
                v = t[:].rearrange("p (g m) -> p g m", m=F8GW)
                g = (r0 + gl * 32) // 32
                seg, lo, hi = f8_seg_of(img, g)
                n = f8_cnt[(img, seg)]
                f8_cnt[(img, seg)] = n + 1
                vv = v[:, gl].rearrange("p (t m) -> p t m", t=2)
                is_stop = n == f8_tot[(img, seg)] - 1
                nc.tensor.matmul(
                    f8_ps[img][seg][:NF8 * 16, :],
                    vv,
                    pred8[:, img * 100 + g, :, :],
                    start=(n == 0),
                    stop=is_stop,
                    perf_mode=mybir.MatmulPerfMode.DoubleRow,
                )
                if is_stop:
                    pend_f8_drains.append((img, seg))

            # deferred PSUM drains: emit the copy late in the producer queue
            # so it never blocks plane-building ops behind its PE wait.
            pend_bf_drains = []   # (img, seg)
            pend_f8_drains = []

            # late drains are split across DVE and Act so they run in
            # parallel at the kernel tail; early ones ride Act (idle then)
            DVE_DRAINS = {LAST_BF, ("f8", IPC - 1, 0)}

            def flush_bf_drains():
                while pend_bf_drains:
                    i, s = pend_bf_drains.pop(0)
                    d0 = DRAIN_AT[("bf", i, s)]
                    if ("bf", i, s) in DVE_DRAINS:
                        nc.vector.tensor_copy(
                            acc[:SLABW, d0:d0 + 32], bf_ps[i][s][:SLABW, :])
                    else:
                        nc.scalar.copy(
                            acc[:SLABW, d0:d0 + 32], bf_ps[i][s][:SLABW, :])

            def flush_f8_drains():
                while pend_f8_drains:
                    i, s = pend_f8_drains.pop(0)
                    d0 = DRAIN_AT[("f8", i, s)]
                    if ("f8", i, s) in DVE_DRAINS:
                        nc.vector.tensor_copy(
                            acc[:NF8 * 16, d0:d0 + 64],
                            f8_ps[i][s][:NF8 * 16, :])
                    else:
                        nc.scalar.copy(
                            acc[:NF8 * 16, d0:d0 + 64],
                            f8_ps[i][s][:NF8 * 16, :])

            for img in range(IPC):
                base = img * FD
                # Act sign planes for this image's act ranges (whole-range
                # ops; the multi-wait splitter absorbs multi-slice label
                # deps)
                for ridx, (c0, c1, kind) in enumerate(F8_RANGES[img]):
                    if kind != "act":
                        continue
                    t = f8_tiles[(img, ridx)]
                    nb = (c1 - c0) // 32
                    v = t[:].rearrange("p (g t k i) -> p g t k i",
                                       g=nb, t=2, k=NF8, i=16)
                    lv = lab_sb[:, base + c0:base + c1].rearrange(
                        "p (g t i) -> p g t i", t=2, i=16)
                    for j in range(NF8):
                        nc.scalar.activation(
                            v[:, :, :, j, :],
                            lv[:],
                            AF.Sign,
                            bias=consts[:, j:j + 1],
                        )

                nspans = len(BF_SPANS[img])
                bship_seen = 0
                for sp, (c0, c1, skind) in enumerate(BF_SPANS[img]):
                    w = c1 - c0
                    gs = w // 8
                    if skind == "bship":
                        slab = bship_tiles[(img, bship_seen)]
                        bship_seen += 1
                        slab3 = slab[:].rearrange("p (g m) -> p g m",
                                                  m=SLABW)
                    else:
                        slab = pool.tile([P, gs * SLABW], BF16,
                                         name=f"slab_{img}_{sp}",
                                         tag="slab", bufs=3)
                        slab3 = slab[:].rearrange("p (g m) -> p g m",
                                                  m=SLABW)
                        lab3 = lab_sb[:, base + c0:base + c1].rearrange(
                            "p (g i) -> p g i", i=8)
                        nc.gpsimd.memset(slab3[:, :, 0:8], 1.0)
                        for j in range(1, NSLOT):
                            k = j + 1
                            ac = _acc_cols[(img, j, sp)]
                            nc.vector.tensor_scalar(
                                out=slab3[:, :, j * 8:(j + 1) * 8],
                                in0=lab3[:],
                                scalar1=float(k),
                                scalar2=None,
                                op0=mybir.AluOpType.is_ge,
                                op1=mybir.AluOpType.add,
                                accum_out=acc[:, ac:ac + 1],
                            )
                    # previous spans' PE work has certainly been consumed by
                    # now; drain those psum segments without stalling DVE/Act
                    flush_bf_drains()
                    flush_f8_drains()
                    # PE: bf16 matmuls for this span
                    for gl in range(gs):
                        g = c0 // 8 + gl
                        seg, lo, hi = bf_seg_of(img, g)
                        nc.tensor.matmul(
                            bf_ps[img][seg][:SLABW, :],
                            slab3[:, gl, :],
                            pred4[:, img * 400 + g, :],
                            start=(g == lo),
                            stop=(g == hi - 1),
                        )
                        if g == hi - 1:
                            pend_bf_drains.append((img, seg))
                    # PE: DoubleRow matmuls for SHIPPED fp8 groups inside
                    # this span (act ranges are deferred to the image tail)
                    for ridx, (r0, r1, kind) in enumerate(F8_RANGES[img]):
                        if kind != "ship":
                            continue
                        for gl in range((r1 - r0) // 32):
                            gc0 = r0 + gl * 32
                            if c0 <= gc0 < c1:
                                emit_f8_group(img, ridx, gl)
                    # act-range f8 matmuls go after a mid/late span's bf
                    # matmuls: the Act engine is done by then and this keeps
                    # them (and the psum close) off the kernel tail
                    if sp == ACT_EMIT_SP[img]:
                        for ridx, (r0, r1, kind) in enumerate(F8_RANGES[img]):
                            if kind != "act":
                                continue
                            for gl in range((r1 - r0) // 32):
                                emit_f8_group(img, ridx, gl)
            flush_bf_drains()
            flush_f8_drains()

            # ---------------- output ----------------
            # main block via Act-HWDGE (Pool's queue is busy with late
            # memsets; SP carries the tail DMA)
            nc.scalar.dma_start(out_d[:, :TAIL_FROM], acc[:, :TAIL_FROM])
            nc.sync.dma_start(out_d[:, TAIL_FROM:], acc[:, TAIL_FROM:])
    return nc


def _get_nc():
    if not _NC_CACHE:
        _NC_CACHE.append(_build_nc())
    return _NC_CACHE[0]


def make_in_maps(pred_similarities, kernel_labels):
    pred = np.ascontiguousarray(pred_similarities, dtype=np.float32).reshape(
        N_CORES, IPC, C, P, FD
    )
    labs0 = np.ascontiguousarray(kernel_labels, dtype=np.int32).reshape(
        N_CORES, IPC, P, FD
    )
    nonbg = labs0 != 0
    pred = pred * nonbg[:, :, None, :, :]
    pred8 = pred.astype(mybir.dt.np(FP8))
    pred8 = pred8.transpose(0, 3, 1, 4, 2).reshape(N_CORES, P, NCOL * C)

    labs16 = labs0.astype(np.float32).astype(ml_dtypes.bfloat16) \
        .transpose(0, 2, 1, 3).reshape(N_CORES, P, NCOL)

    # shipped fp8 sign planes, interleaved (group, ktile, k, i)
    shipw = max(SHIP_ARENA, 64)
    ship = np.zeros((N_CORES, P, shipw), mybir.dt.np(FP8))
    off = 0
    for img in range(IPC):
        for c0, c1, kind in F8_RANGES[img]:
            if kind != "ship":
                continue
            w = c1 - c0
            lr = labs0[:, img, :, c0:c1]                       # [cores,P,w]
            sgn = np.zeros((N_CORES, P, w, NF8), np.float32)
            for j in range(NF8):
                sgn[..., j] = (lr >= F8K0 + j) * 2.0 - 1.0
            sgn = sgn.reshape(N_CORES, P, w // 32, 2, 16, NF8)
            sgn = sgn.transpose(0, 1, 2, 3, 5, 4).reshape(N_CORES, P, w * NF8)
            ship[:, :, off:off + w * NF8] = sgn.astype(mybir.dt.np(FP8))
            off += w * NF8

    # host-shipped interleaved bf16 slabs (ones slot + 0/1 step planes)
    bshipw = max(BSHIP_ARENA, 64)
    bship = np.ones((N_CORES, P, bshipw), np.float32)
    boff = 0
    for img in range(IPC):
        for c0, c1 in BSHIP_SPANS[img]:
            w = c1 - c0
            lr = labs0[:, img, :, c0:c1]
            blk = np.ones((N_CORES, P, w // 8, NSLOT, 8), np.float32)
            for j in range(1, NSLOT):
                blk[:, :, :, j, :] = (lr >= j + 1).reshape(
                    N_CORES, P, w // 8, 8)
            n = w // 8 * SLABW
            bship[:, :, boff:boff + n] = blk.reshape(N_CORES, P, n)
            boff += n
    bship16 = bship.astype(ml_dtypes.bfloat16)

    # host-side counts: c>=1 (background mask is already computed for pred
    # zeroing), c>=k for the fp8 slots, and the bship ranges' share of the
    # DVE slots
    host_cge = np.zeros((N_CORES, IPC, K1 + 1), np.float64)
    host_cge[:, :, 1] = nonbg.sum(axis=(2, 3))
    for k in range(F8K0, K1):
        host_cge[:, :, k] = (labs0 >= k).sum(axis=(2, 3))
    host_bk = np.zeros((N_CORES, IPC, K1 + 1), np.float64)
    for img in range(IPC):
        for c0, c1 in BSHIP_SPANS[img]:
            lr = labs0[:, img, :, c0:c1]
            for k in range(2, 2 + D):
                host_bk[:, img, k] += (lr >= k).sum(axis=(1, 2))

    in_maps = [
        {"pred": np.ascontiguousarray(pred8[i]),
         "lab": np.ascontiguousarray(labs16[i]),
         "ship": np.ascontiguousarray(ship[i]),
         "bship": np.ascontiguousarray(bship16[i])}
        for i in range(N_CORES)
    ]
    return in_maps, host_cge, host_bk


def kernel(pred_similarities, kernel_labels):
    global LAST_RESULT
    nc = _get_nc()
    in_maps, host_cge, host_bk = make_in_maps(pred_similarities, kernel_labels)
    res = run_bass_kernel_spmd(nc, in_maps, core_ids=list(range(N_CORES)), **RUN_KWARGS)
    LAST_RESULT = res
    outs = [np.asarray(res.results[c]["out"]) for c in range(N_CORES)]
    return epilogue(outs, host_cge, host_bk)


def epilogue(outs, host_cge, host_bk):
    S = np.zeros((B, K1, C), np.float64)
    counts = np.zeros((B, K1), np.float64)
    for core in range(N_CORES):
        o = outs[core].astype(np.float64)
        for img in range(IPC):
            b = core * IPC + img
            Fbf = np.zeros((NSLOT, C))
            for s in range(len(BF_SEGS[img])):
                d0 = DRAIN_AT[("bf", img, s)]
                ps = o[:SLABW, d0:d0 + 32].reshape(NSLOT, 8, 8, C)
                Fbf += np.einsum("siic->sc", ps)
            F8 = np.zeros((NF8, C))
            for s in range(len(F8_SEGS[img])):
                d0 = DRAIN_AT[("f8", img, s)]
                ps = o[:NF8 * 16, d0:d0 + 64].reshape(NF8, 16, 16, C)
                F8 += np.einsum("siic->sc", ps)
            s_ge = np.zeros((K1 + 2, C))
            s_ge[1] = Fbf[0]
            for j in range(1, NSLOT):
                s_ge[j + 1] = Fbf[j]
            for j in range(NF8):
                s_ge[F8K0 + j] = (F8[j] + s_ge[1]) / 2.0
            c_ge = np.zeros(K1 + 2)
            c_ge[1] = host_cge[core, img, 1]
            for j in range(1, NSLOT):
                tot = host_bk[core, img, j + 1]
                for sp in range(len(BF_SPANS[img])):
                    if (img, j, sp) in _acc_cols:
                        tot += o[:, _acc_cols[(img, j, sp)]].sum()
                c_ge[j + 1] = tot
            for k in range(F8K0, K1):
                c_ge[k] = host_cge[core, img, k]
            S[b, 1:, :] = s_ge[1:K1] - s_ge[2:K1 + 1]
            counts[b, 1:] = c_ge[1:K1] - c_ge[2:K1 + 1]
            counts[b, 0] = HW - c_ge[1]

    N = np.linalg.norm(S, axis=-1)
    N[:, 0] = 0.0
    f = np.log(np.maximum(SIGMA_DIS - N, 0.0) ** 2 + 1.0)
    sum_g = (counts * f).sum(axis=-1)
    present = counts > 0.5
    Kb = np.where(
        present.any(axis=1), (present * np.arange(K1)).max(axis=1), 0
    ).astype(np.float64)
    active = Kb > 1.0
    Pn = Kb * (Kb - 1.0) * 0.5
    own = np.where(active, (Kb - 1.0) * sum_g + HW * (Pn - (Kb - 1.0)) * F0, 0.0)
    P_act = np.where(active, Pn, 0.0)
    other = (P_act.sum() - P_act) * HW * F0
    scale = np.where(active, 1.0 / (Kb * (Kb - 1.0)), Kb)
    return np.float32((scale * (own + other)).sum())


# revision 3
# speedup vs baseline: 1.0016x; 1.0016x over previous
"""Trainium2 Bass kernel for nn_DiscriminationLoss (segment_reduce), v2.

Math: per image b the loss reduces to
  S[b,k,c]    = sum of pred[b,c] over pixels with label k   (k=1..16)
  counts[b,k] = histogram of labels
followed by a tiny scalar epilogue (norms, log-relu pair combination).

Device strategy (2 images per core, data-parallel over batch):
- Pixels: [128 partitions, 3200 pixel-cols] per image.  All step
  functionals S>=k = sum_{lab>=k} pred come from PE matmuls against
  per-pixel mask planes; S_k = S>=k - S>=k+1 on the host.
- bf16 main pass: stationary slab interleaved per 8-pixel-col group as
  11 slots x 8 chunks = 88 cols:
    slot 0  = all-ones plane  -> S>=1  (pred is host-zeroed on
              background, so the sum over ALL pixels = sum over lab>=1).
              Built by gpsimd/Pool MEMSET - zero DVE/Act cost.
    slots 1..10 = [lab >= k], k=2..11, built by DVE tensor_scalar
              (is_ge) in 4x perf mode, with free per-partition count
              accumulation (accum_out).
- fp8 side pass: 5 sign planes Sign(lab-k+0.5) = +-1 for k=12..16, in
  an interleaved (group, ktile, k, i) fp8 arena consumed by PE
  DoubleRow matmuls (256-deep contraction at 0.5 cyc/row).  Column
  ranges are either Act-built (Sign activation, bf16 in / fp8 out) or
  host-shipped verbatim via DMA (+-1 is exact in fp8e4m3).
  Decode: F8 = sum(+-1 * pred) = 2*S>=k - S>=1.
- Counts: k=2..11 from DVE accum_out; k=1 and k>=12 host-side from the
  label histogram (the background mask is already computed on the host
  for pred zeroing, so counting is free there).
- pred is fp8e4m3, host-converted and background-zeroed; labels bf16.

Toolchain notes:
- walrus rejects multi-wait instructions and any wait on TensorScalar;
  the BIR is post-processed to move those onto Drain predecessors.
- Matmul stationary APs must have ONE free dim (two for DoubleRow with
  16B-aligned even strides) - hence the interleaved slab layouts.

Sharding: data-parallel over batch, 2 images per core, no collectives.
"""

import json

import numpy as np
import ml_dtypes

import concourse.bass as bass
import concourse.mybir as mybir
import concourse.tile as tile
import concourse.bass2jax as _b2j
from concourse.bass_utils import run_bass_kernel_spmd


def _split_multiwait_bir(bir_json: bytes) -> bytes:
    """walrus rejects instructions carrying more than one sync wait, and any
    wait riding a TensorScalarPtr; split extra waits onto single-wait
    same-engine Drain predecessors."""
    d = json.loads(bir_json)
    changed = False
    for fn in d.get("functions", []):
        for bb in fn.get("blocks", []):
            insts = bb.get("instructions", [])
            out = []
            for ins in insts:
                si = ins.get("sync_info") or {}
                waits = si.get("on_wait") or []
                keep = 0 if (waits and ins.get("opcode") == "TensorScalarPtr") \
                    else 1
                if len(waits) > keep:
                    changed = True
                    split = waits[:len(waits) - keep]
                    for wi, w in enumerate(split):
                        out.append(
                            {
                                "debug": ins.get("debug"),
                                "engine": ins["engine"],
                                "ins": [],
                                "is_reset_sema": False,
                                "name": f"{ins['name']}_w{wi}",
                                "opcode": "Drain",
                                "outs": [],
                                "sync_info": {"on_update": [], "on_wait": [w]},
                            }
                        )
                    si["on_wait"] = waits[len(waits) - keep:]
                out.append(ins)
            bb["instructions"] = out
    if not changed:
        return bir_json
    return json.dumps(d).encode()


_ORIG_COMPILE_BIR = _b2j.compile_bir_kernel


def _compile_bir_splitting_waits(bir_json, tmpdir, neff_name="file.neff"):
    return _ORIG_COMPILE_BIR(_split_multiwait_bir(bir_json), tmpdir, neff_name=neff_name)


_b2j.compile_bir_kernel = _compile_bir_splitting_waits

B, C, H, W = 16, 4, 640, 640
HW = H * W
P = 128
FD = HW // P               # 3200 pixel-cols per image
N_CORES = 8
IPC = B // N_CORES         # images per core
NCOL = IPC * FD            # 6400 pixel-cols per core
KMAX = 16
K1 = KMAX + 1
SIGMA_DIS = 3.0
F0 = float(np.log(SIGMA_DIS**2 + 1.0))

# ---------------- configuration ----------------
D = 10                      # DVE step slots, k = 2..11
NSLOT = 1 + D               # + ones slot
SLABW = NSLOT * 8           # 88 stationary cols per 8-pixel-col group
NF8 = 5                     # fp8 sign slots, k = 12..16
F8K0 = 2 + D                # first fp8 k (=12)
F8GW = 2 * NF8 * 16         # fp8 arena cols per 32-pixel-col group (=160)

# bf16 spans per image (pixel-col boundaries); each span = one slab ring
# tile, one DVE op per step slot, one Pool ones-memset.
# spans: (c0, c1, kind) with kind "dve" (device-built planes) or "bship"
# (host-shipped interleaved bf16 slab; PE-only, no DVE/Act work)
BF_SPANS = [
    [(0, 384, "dve"), (384, 1920, "dve"), (1920, 3200, "dve")],
    [(0, 640, "dve"), (640, 1920, "dve"), (1920, 2880, "dve"),
     (2880, 3136, "dve"), (3136, 3200, "dve")],
]
# label DMA slices per image (pixel-cols)
LAB_SLICES = [
    [(0, 384), (384, 1920), (1920, 3200)],
    [(0, 640), (640, 1920), (1920, 3200)],
]
# emit act-range DoubleRow matmuls after this span's bf matmuls
ACT_EMIT_SP = {0: 1, 1: 2}
# accums of these (img, span) and drains of these segs close late -> tail
LATE_SPANS = {(1, 2), (1, 3), (1, 4)}
LATE_SEGS = {("bf", 1, 1), ("bf", 1, 2), ("f8", 1, 0), ("f8", 1, 1)}
# fp8 ranges per image: (c0, c1, kind); 32-col aligned
F8_RANGES = [
    [(0, 1920, "act"), (1920, 3200, "ship")],
    [(0, 1408, "ship"), (1408, 3072, "act"), (3072, 3200, "ship")],
]
# psum segments per image for the bf16 pass (8-col group indices);
# boundaries must coincide with span ends
BF_SEGS = [
    [(0, 240), (240, 400)],
    [(0, 240), (240, 392), (392, 400)],
]
# psum segments per image for the fp8 pass (32-col group indices)
F8_SEGS = [
    [(0, 100)],
    [(0, 98), (98, 100)],
]
# pred DMA slices per image (pixel-cols)
PRED_SLICES = [
    [(0, 384), (384, 1920), (1920, 3200)],
    [(0, 640), (640, 1920), (1920, 3200)],
]
# DMA issue order: tokens (kind, img, idx); labels early, pred/ship behind
DMA_ORDER = [
    ("lab", 0, 0), ("lab", 0, 1), ("pred", 0, 0), ("lab", 0, 2),
    ("lab", 1, 0), ("pred", 0, 1), ("ship", 0, 0), ("lab", 1, 1),
    ("lab", 1, 2), ("pred", 0, 2), ("pred", 1, 0), ("ship", 1, 0),
    ("pred", 1, 1), ("ship", 1, 1), ("pred", 1, 2),
]

LAST_BF = ("bf", IPC - 1, len(BF_SEGS[IPC - 1]) - 1)
LAST_F8 = ("f8", IPC - 1, len(F8_SEGS[IPC - 1]) - 1)

RUN_KWARGS = {}
LAST_RESULT = None
_NC_CACHE = []

BF16 = mybir.dt.bfloat16
FP8 = mybir.dt.float8e4
F32 = mybir.dt.float32
AF = mybir.ActivationFunctionType

SHIP_COLS = [[(c0, c1) for c0, c1, kind in F8_RANGES[i] if kind == "ship"]
             for i in range(IPC)]
SHIP_TOT = sum(c1 - c0 for i in range(IPC) for c0, c1 in SHIP_COLS[i])
SHIP_ARENA = SHIP_TOT * NF8
BSHIP_SPANS = [[(c0, c1) for c0, c1, kind in BF_SPANS[i] if kind == "bship"]
               for i in range(IPC)]
BSHIP_ARENA = sum((c1 - c0) // 8 * SLABW
                  for i in range(IPC) for c0, c1 in BSHIP_SPANS[i])

# --- out tensor layout: [early accums | early drains | tail] ---
# accums/drains that close late live in the tail region so the main
# out-DMA never waits on them.
_acc_cols = {}
DRAIN_AT = {}
_c = 0
for i in range(IPC):
    for sp in range(len(BF_SPANS[i])):
        if (i, sp) in LATE_SPANS or BF_SPANS[i][sp][2] != "dve":
            continue
        for j in range(1, NSLOT):
            _acc_cols[(i, j, sp)] = _c
            _c += 1
_c = (_c + 31) // 32 * 32
for i in range(IPC):
    for s in range(len(BF_SEGS[i])):
        if ("bf", i, s) not in LATE_SEGS:
            DRAIN_AT[("bf", i, s)] = _c
            _c += 32
    for s in range(len(F8_SEGS[i])):
        if ("f8", i, s) not in LATE_SEGS:
            DRAIN_AT[("f8", i, s)] = _c
            _c += 64
TAIL_FROM = _c
for i in range(IPC):
    for sp in range(len(BF_SPANS[i])):
        if (i, sp) not in LATE_SPANS or BF_SPANS[i][sp][2] != "dve":
            continue
        for j in range(1, NSLOT):
            _acc_cols[(i, j, sp)] = _c
            _c += 1
_c = (_c + 31) // 32 * 32
for kind, i, s in sorted(LATE_SEGS):
    DRAIN_AT[(kind, i, s)] = _c
    _c += 32 if kind == "bf" else 64
# pad the tail block to >=128 f32 cols so its DMA runs at full descriptor
# bandwidth (contiguous run >= 512B per partition)
if _c - TAIL_FROM < 128:
    _c = TAIL_FROM + 128
OUT_COLS = _c


def _build_nc():
    nc = bass.Bass("TRN2", target_bir_lowering=False, debug=False)
    pred_d = nc.dram_tensor("pred", [P, NCOL * C], FP8, kind="ExternalInput")
    lab_d = nc.dram_tensor("lab", [P, NCOL], BF16, kind="ExternalInput")
    ship_d = nc.dram_tensor("ship", [P, max(SHIP_ARENA, 64)], FP8,
                            kind="ExternalInput")
    bship_d = nc.dram_tensor("bship", [P, max(BSHIP_ARENA, 64)], BF16,
                             kind="ExternalInput")
    out_d = nc.dram_tensor("out", [P, OUT_COLS], F32, kind="ExternalOutput")

    with tile.TileContext(nc) as tc:
        with tc.tile_pool(name="pool", bufs=1) as pool, \
             tc.tile_pool(name="ps", bufs=1, space="PSUM") as pspool:
            pred_sb = pool.tile([P, NCOL * C], FP8, name="pred_sb")
            lab_sb = pool.tile([P, NCOL], BF16, name="lab_sb")
            acc = pool.tile([P, OUT_COLS], F32, name="acc")
            consts = pool.tile([P, NF8], F32, name="consts")
            for j in range(NF8):
                nc.gpsimd.memset(consts[:, j:j + 1], 0.5 - float(F8K0 + j))

            # fp8 arena tiles, one per range
            f8_tiles = {}
            for img in range(IPC):
                for ridx, (c0, c1, kind) in enumerate(F8_RANGES[img]):
                    f8_tiles[(img, ridx)] = pool.tile(
                        [P, (c1 - c0) * NF8], FP8, name=f"f8_{img}_{ridx}")
            # host-shipped bf16 slab tiles (flat DMA destinations)
            bship_tiles = {}
            for img in range(IPC):
                for bidx, (c0, c1) in enumerate(BSHIP_SPANS[img]):
                    bship_tiles[(img, bidx)] = pool.tile(
                        [P, (c1 - c0) // 8 * SLABW], BF16,
                        name=f"bs_{img}_{bidx}")

            # ---------------- DMA schedule ----------------
            ship_ranges = []   # (img, ridx) in ship order
            for img in range(IPC):
                for ridx, (c0, c1, kind) in enumerate(F8_RANGES[img]):
                    if kind == "ship":
                        ship_ranges.append((img, ridx))
            ship_offs = {}
            off = 0
            for img, ridx in ship_ranges:
                c0, c1, _ = F8_RANGES[img][ridx]
                ship_offs[(img, ridx)] = off
                off += (c1 - c0) * NF8
            ship_by_img = {}
            for img, ridx in ship_ranges:
                ship_by_img.setdefault(img, []).append(ridx)
            bship_offs = {}
            boff = 0
            for img in range(IPC):
                for bidx, (c0, c1) in enumerate(BSHIP_SPANS[img]):
                    bship_offs[(img, bidx)] = boff
                    boff += (c1 - c0) // 8 * SLABW

            for kind, img, idx in DMA_ORDER:
                base = img * FD
                if kind == "lab":
                    c0, c1 = LAB_SLICES[img][idx]
                    nc.sync.dma_start(lab_sb[:, base + c0:base + c1],
                                      lab_d[:, base + c0:base + c1])
                elif kind == "pred":
                    c0, c1 = PRED_SLICES[img][idx]
                    nc.sync.dma_start(
                        pred_sb[:, (base + c0) * 4:(base + c1) * 4],
                        pred_d[:, (base + c0) * 4:(base + c1) * 4])
                elif kind == "ship":
                    ridx = ship_by_img[img][idx]
                    t = f8_tiles[(img, ridx)]
                    o = ship_offs[(img, ridx)]
                    w = t.shape[1]
                    nc.sync.dma_start(t[:], ship_d[:, o:o + w])
                elif kind == "bslab":
                    t = bship_tiles[(img, idx)]
                    o = bship_offs[(img, idx)]
                    w = t.shape[1]
                    nc.sync.dma_start(t[:], bship_d[:, o:o + w])

            # moving views
            pred4 = pred_sb[:].rearrange("p (g m) -> p g m", m=32)
            pred8 = pred_sb[:].rearrange("p (g t m) -> p g t m", t=2, m=64)

            bf_ps = [[pspool.tile([P, 32], F32, name=f"bps_{i}_{s}")
                      for s in range(len(BF_SEGS[i]))] for i in range(IPC)]
            f8_ps = [[pspool.tile([P, 64], F32, name=f"fps_{i}_{s}")
                      for s in range(len(F8_SEGS[i]))] for i in range(IPC)]

            def bf_seg_of(img, g):
                for s, (lo, hi) in enumerate(BF_SEGS[img]):
                    if lo <= g < hi:
                        return s, lo, hi
                raise AssertionError

            def f8_seg_of(img, g):
                for s, (lo, hi) in enumerate(F8_SEGS[img]):
                    if lo <= g < hi:
                        return s, lo, hi
                raise AssertionError

            # fp8 matmul bookkeeping: start/stop flags follow EMISSION order
            # (PE executes in order), so count totals per (img, seg) first.
            f8_tot = {}
            f8_cnt = {}
            for img in range(IPC):
                for s, (lo, hi) in enumerate(F8_SEGS[img]):
                    f8_tot[(img, s)] = hi - lo
                    f8_cnt[(img, s)] = 0

            def emit_f8_group(img, ridx, gl):
                r0, r1, kind = F8_RANGES[img][ridx]
                t = f8_tiles[(img, ridx)]
                nb = (r1 - r0) // 32
                v = t[:].rearrange("p (g m) -> p g m", m=F8GW)
                g = (r0 + gl * 32) // 32
                seg, lo, hi = f8_seg_of(img, g)
                n = f8_cnt[(img, seg)]
                f8_cnt[(img, seg)] = n + 1
                vv = v[:, gl].rearrange("p (t m) -> p t m", t=2)
                is_stop = n == f8_tot[(img, seg)] - 1
                nc.tensor.matmul(
                    f8_ps[img][seg][:NF8 * 16, :],
                    vv,
                    pred8[:, img * 100 + g, :, :],
                    start=(n == 0),
                    stop=is_stop,
                    perf_mode=mybir.MatmulPerfMode.DoubleRow,
                )
                if is_stop:
                    pend_f8_drains.append((img, seg))

            # deferred PSUM drains: emit the copy late in the producer queue
            # so it never blocks plane-building ops behind its PE wait.
            pend_bf_drains = []   # (img, seg)
            pend_f8_drains = []

            # late drains are split across DVE and Act so they run in
            # parallel at the kernel tail; early ones ride Act (idle then)
            DVE_DRAINS = {LAST_BF, ("f8", IPC - 1, 0)}

            def flush_bf_drains():
                while pend_bf_drains:
                    i, s = pend_bf_drains.pop(0)
                    d0 = DRAIN_AT[("bf", i, s)]
                    if ("bf", i, s) in DVE_DRAINS:
                        nc.vector.tensor_copy(
                            acc[:SLABW, d0:d0 + 32], bf_ps[i][s][:SLABW, :])
                    else:
                        nc.scalar.copy(
                            acc[:SLABW, d0:d0 + 32], bf_ps[i][s][:SLABW, :])

            def flush_f8_drains():
                while pend_f8_drains:
                    i, s = pend_f8_drains.pop(0)
                    d0 = DRAIN_AT[("f8", i, s)]
                    if ("f8", i, s) in DVE_DRAINS:
                        nc.vector.tensor_copy(
                            acc[:NF8 * 16, d0:d0 + 64],
                            f8_ps[i][s][:NF8 * 16, :])
                    else:
                        nc.scalar.copy(
                            acc[:NF8 * 16, d0:d0 + 64],
                            f8_ps[i][s][:NF8 * 16, :])

            for img in range(IPC):
                base = img * FD
                # Act sign planes for this image's act ranges (whole-range
                # ops; the multi-wait splitter absorbs multi-slice label
                # deps)
                for ridx, (c0, c1, kind) in enumerate(F8_RANGES[img]):
                    if kind != "act":
                        continue
                    t = f8_tiles[(img, ridx)]
                    nb = (c1 - c0) // 32
                    v = t[:].rearrange("p (g t k i) -> p g t k i",
                                       g=nb, t=2, k=NF8, i=16)
                    lv = lab_sb[:, base + c0:base + c1].rearrange(
                        "p (g t i) -> p g t i", t=2, i=16)
                    for j in range(NF8):
                        nc.scalar.activation(
                            v[:, :, :, j, :],
                            lv[:],
                            AF.Sign,
                            bias=consts[:, j:j + 1],
                        )

                nspans = len(BF_SPANS[img])
                bship_seen = 0
                for sp, (c0, c1, skind) in enumerate(BF_SPANS[img]):
                    w = c1 - c0
                    gs = w // 8
                    if skind == "bship":
                        slab = bship_tiles[(img, bship_seen)]
                        bship_seen += 1
                        slab3 = slab[:].rearrange("p (g m) -> p g m",
                                                  m=SLABW)
                    else:
                        slab = pool.tile([P, gs * SLABW], BF16,
                                         name=f"slab_{img}_{sp}",
                                         tag="slab", bufs=3)
                        slab3 = slab[:].rearrange("p (g m) -> p g m",
                                                  m=SLABW)
                        lab3 = lab_sb[:, base + c0:base + c1].rearrange(
                            "p (g i) -> p g i", i=8)
                        nc.gpsimd.memset(slab3[:, :, 0:8], 1.0)
                        for j in range(1, NSLOT):
                            k = j + 1
                            ac = _acc_cols[(img, j, sp)]
                            nc.vector.tensor_scalar(
                                out=slab3[:, :, j * 8:(j + 1) * 8],
                                in0=lab3[:],
                                scalar1=float(k),
                                scalar2=None,
                                op0=mybir.AluOpType.is_ge,
                                op1=mybir.AluOpType.add,
                                accum_out=acc[:, ac:ac + 1],
                            )
                    # previous spans' PE work has certainly been consumed by
                    # now; drain those psum segments without stalling DVE/Act
                    flush_bf_drains()
                    flush_f8_drains()
                    # PE: bf16 matmuls for this span
                    for gl in range(gs):
                        g = c0 // 8 + gl
                        seg, lo, hi = bf_seg_of(img, g)
                        nc.tensor.matmul(
                            bf_ps[img][seg][:SLABW, :],
                            slab3[:, gl, :],
                            pred4[:, img * 400 + g, :],
                            start=(g == lo),
                            stop=(g == hi - 1),
                        )
                        if g == hi - 1:
                            pend_bf_drains.append((img, seg))
                    # PE: DoubleRow matmuls for SHIPPED fp8 groups inside
                    # this span (act ranges are deferred to the image tail)
                    for ridx, (r0, r1, kind) in enumerate(F8_RANGES[img]):
                        if kind != "ship":
                            continue
                        for gl in range((r1 - r0) // 32):
                            gc0 = r0 + gl * 32
                            if c0 <= gc0 < c1:
                                emit_f8_group(img, ridx, gl)
                    # act-range f8 matmuls go after a mid/late span's bf
                    # matmuls: the Act engine is done by then and this keeps
                    # them (and the psum close) off the kernel tail
                    if sp == ACT_EMIT_SP[img]:
                        for ridx, (r0, r1, kind) in enumerate(F8_RANGES[img]):
                            if kind != "act":
                                continue
                            for gl in range((r1 - r0) // 32):
                                emit_f8_group(img, ridx, gl)
            flush_bf_drains()
            flush_f8_drains()

            # ---------------- output ----------------
            # main block via Act-HWDGE (Pool's queue is busy with late
            # memsets; SP carries the tail DMA)
            nc.scalar.dma_start(out_d[:, :TAIL_FROM], acc[:, :TAIL_FROM])
            nc.sync.dma_start(out_d[:, TAIL_FROM:], acc[:, TAIL_FROM:])
    return nc


def _get_nc():
    if not _NC_CACHE:
        _NC_CACHE.append(_build_nc())
    return _NC_CACHE[0]


def make_in_maps(pred_similarities, kernel_labels):
    pred = np.ascontiguousarray(pred_similarities, dtype=np.float32).reshape(
        N_CORES, IPC, C, P, FD
    )
    labs0 = np.ascontiguousarray(kernel_labels, dtype=np.int32).reshape(
        N_CORES, IPC, P, FD
    )
    nonbg = labs0 != 0
    pred = pred * nonbg[:, :, None, :, :]
    pred8 = pred.astype(mybir.dt.np(FP8))
    pred8 = pred8.transpose(0, 3, 1, 4, 2).reshape(N_CORES, P, NCOL * C)

    labs16 = labs0.astype(np.float32).astype(ml_dtypes.bfloat16) \
        .transpose(0, 2, 1, 3).reshape(N_CORES, P, NCOL)

    # shipped fp8 sign planes, interleaved (group, ktile, k, i)
    shipw = max(SHIP_ARENA, 64)
    ship = np.zeros((N_CORES, P, shipw), mybir.dt.np(FP8))
    off = 0
    for img in range(IPC):
        for c0, c1, kind in F8_RANGES[img]:
            if kind != "ship":
                continue
            w = c1 - c0
            lr = labs0[:, img, :, c0:c1]                       # [cores,P,w]
            sgn = np.zeros((N_CORES, P, w, NF8), np.float32)
            for j in range(NF8):
                sgn[..., j] = (lr >= F8K0 + j) * 2.0 - 1.0
            sgn = sgn.reshape(N_CORES, P, w // 32, 2, 16, NF8)
            sgn = sgn.transpose(0, 1, 2, 3, 5, 4).reshape(N_CORES, P, w * NF8)
            ship[:, :, off:off + w * NF8] = sgn.astype(mybir.dt.np(FP8))
            off += w * NF8

    # host-shipped interleaved bf16 slabs (ones slot + 0/1 step planes)
    bshipw = max(BSHIP_ARENA, 64)
    bship = np.ones((N_CORES, P, bshipw), np.float32)
    boff = 0
    for img in range(IPC):
        for c0, c1 in BSHIP_SPANS[img]:
            w = c1 - c0
            lr = labs0[:, img, :, c0:c1]
            blk = np.ones((N_CORES, P, w // 8, NSLOT, 8), np.float32)
            for j in range(1, NSLOT):
                blk[:, :, :, j, :] = (lr >= j + 1).reshape(
                    N_CORES, P, w // 8, 8)
            n = w // 8 * SLABW
            bship[:, :, boff:boff + n] = blk.reshape(N_CORES, P, n)
            boff += n
    bship16 = bship.astype(ml_dtypes.bfloat16)

    # host-side counts: c>=1 (background mask is already computed for pred
    # zeroing), c>=k for the fp8 slots, and the bship ranges' share of the
    # DVE slots
    host_cge = np.zeros((N_CORES, IPC, K1 + 1), np.float64)
    host_cge[:, :, 1] = nonbg.sum(axis=(2, 3))
    for k in range(F8K0, K1):
        host_cge[:, :, k] = (labs0 >= k).sum(axis=(2, 3))
    host_bk = np.zeros((N_CORES, IPC, K1 + 1), np.float64)
    for img in range(IPC):
        for c0, c1 in BSHIP_SPANS[img]:
            lr = labs0[:, img, :, c0:c1]
            for k in range(2, 2 + D):
                host_bk[:, img, k] += (lr >= k).sum(axis=(1, 2))

    in_maps = [
        {"pred": np.ascontiguousarray(pred8[i]),
         "lab": np.ascontiguousarray(labs16[i]),
         "ship": np.ascontiguousarray(ship[i]),
         "bship": np.ascontiguousarray(bship16[i])}
        for i in range(N_CORES)
    ]
    return in_maps, host_cge, host_bk


def kernel(pred_similarities, kernel_labels):
    global LAST_RESULT
    nc = _get_nc()
    in_maps, host_cge, host_bk = make_in_maps(pred_similarities, kernel_labels)
    res = run_bass_kernel_spmd(nc, in_maps, core_ids=list(range(N_CORES)), **RUN_KWARGS)
    LAST_RESULT = res
    outs = [np.asarray(res.results[c]["out"]) for c in range(N_CORES)]
    return epilogue(outs, host_cge, host_bk)


def epilogue(outs, host_cge, host_bk):
    S = np.zeros((B, K1, C), np.float64)
    counts = np.zeros((B, K1), np.float64)
    for core in range(N_CORES):
        o = outs[core].astype(np.float64)
        for img in range(IPC):
            b = core * IPC + img
            Fbf = np.zeros((NSLOT, C))
            for s in range(len(BF_SEGS[img])):
                d0 = DRAIN_AT[("bf", img, s)]
                ps = o[:SLABW, d0:d0 + 32].reshape(NSLOT, 8, 8, C)
                Fbf += np.einsum("siic->sc", ps)
            F8 = np.zeros((NF8, C))
            for s in range(len(F8_SEGS[img])):
                d0 = DRAIN_AT[("f8", img, s)]
                ps = o[:NF8 * 16, d0:d0 + 64].reshape(NF8, 16, 16, C)
                F8 += np.einsum("siic->sc", ps)
            s_ge = np.zeros((K1 + 2, C))
            s_ge[1] = Fbf[0]
            for j in range(1, NSLOT):
                s_ge[j + 1] = Fbf[j]
            for j in range(NF8):
                s_ge[F8K0 + j] = (F8[j] + s_ge[1]) / 2.0
            c_ge = np.zeros(K1 + 2)
            c_ge[1] = host_cge[core, img, 1]
            for j in range(1, NSLOT):
                tot = host_bk[core, img, j + 1]
                for sp in range(len(BF_SPANS[img])):
                    if (img, j, sp) in _acc_cols:
                        tot += o[:, _acc_cols[(img, j, sp)]].sum()
                c_ge[j + 1] = tot
            for k in range(F8K0, K1):
                c_ge[k] = host_cge[core, img, k]
            S[b, 1:, :] = s_ge[1:K1] - s_ge[2:K1 + 1]
            counts[b, 1:] = c_ge[1:K1] - c_ge[2:K1 + 1]
            counts[b, 0] = HW - c_ge[1]

    N = np.linalg.norm(S, axis=-1)
    N[:, 0] = 0.0
    f = np.log(np.maximum(SIGMA_DIS - N, 0.0) ** 2 + 1.0)
    sum_g = (counts * f).sum(axis=-1)
    present = counts > 0.5
    Kb = np.where(
        present.any(axis=1), (present * np.arange(K1)).max(axis=1), 0
    ).astype(np.float64)
    active = Kb > 1.0
    Pn = Kb * (Kb - 1.0) * 0.5
    own = np.where(active, (Kb - 1.0) * sum_g + HW * (Pn - (Kb - 1.0)) * F0, 0.0)
    P_act = np.where(active, Pn, 0.0)
    other = (P_act.sum() - P_act) * HW * F0
    scale = np.where(active, 1.0 / (Kb * (Kb - 1.0)), Kb)
    return np.float32((scale * (own + other)).sum())
